# revision 1
# baseline (speedup 1.0000x reference)
"""Fallback: chunked bf16 kernel with ncfw AllGathers (r1) + tail trims.

Same as the 385us r1 kernel, plus: chunk-1 normalize on the fast ACT+DVE
path (stream is over, DVE is idle), and chunk-1 collective-adjacent DMAs
on the sync HWDGE ring (free after streaming ends, lower fixed cost than
SWDGE)."""

import sys

if "/opt/trn_rl_repo" not in sys.path:
    sys.path.insert(0, "/opt/trn_rl_repo")

import numpy as np

B_FULL = 512
C_IN = 2048
T_POOL = 196
O_OUT = 512
N_CORES = 8

N_CHUNKS = 2


def build_kernel(b_full, c_in, t_pool, o_out, n_cores, ft_bufs=8):
    import concourse.mybir as mybir
    import concourse.tile as tile
    from concourse import bacc
    from concourse.masks import make_identity

    f32 = mybir.dt.float32
    bf16 = mybir.dt.bfloat16
    AL = mybir.AluOpType
    AF = mybir.ActivationFunctionType
    X = mybir.AxisListType.X

    bc = b_full // n_cores
    nj = 16
    ck = bc // N_CHUNKS
    oc = o_out // 128
    nr = n_cores
    assert c_in == 128 * nj and bc % N_CHUNKS == 0 and o_out % 128 == 0

    nc = bacc.Bacc("TRN2", target_bir_lowering=False, debug=False,
                   enable_asserts=False, num_devices=n_cores)
    feat = nc.dram_tensor("features", [bc, c_in, t_pool], f32,
                          kind="ExternalInput").ap()
    w_in = nc.dram_tensor("w", [o_out, c_in], f32, kind="ExternalInput").ap()
    bias_in = nc.dram_tensor("bias", [1, o_out], f32, kind="ExternalInput").ap()
    out_d = nc.dram_tensor("out", [bc, b_full], f32, kind="ExternalOutput").ap()

    with tile.TileContext(nc) as tc:
        with (
            tc.tile_pool(name="const", bufs=1) as constp,
            tc.tile_pool(name="wload", bufs=1) as wlp,
            tc.tile_pool(name="wtp", bufs=1) as wtp,
            tc.tile_pool(name="featp", bufs=ft_bufs) as fp,
            tc.tile_pool(name="featl", bufs=ft_bufs) as flp,
            tc.tile_pool(name="poolp", bufs=1) as lp,
            tc.tile_pool(name="postp", bufs=1) as pp,
            tc.tile_pool(name="psrot", bufs=2, space="PSUM") as psp,
            tc.tile_pool(name="psgps", bufs=2, space="PSUM") as psgp,
            tc.tile_pool(name="pssim", bufs=1, space="PSUM") as pssp,
            tc.tile_pool(name="pssimh", bufs=1, space="PSUM") as pssp2,
            tc.tile_pool(name="dram", bufs=1, space="DRAM") as dp,
        ):
            # ---- constants ----
            identf = constp.tile([128, 128], f32, name="identf")
            make_identity(nc, identf)
            identb = constp.tile([ck, ck], bf16, name="identb")
            make_identity(nc, identb)
            ones = constp.tile([1, ck], bf16, name="ones")
            nc.vector.memset(ones, 1.0)
            bias_sb = constp.tile([1, o_out], f32, name="bias_sb")
            nc.sync.dma_start(bias_sb[:], bias_in[:])
            bias_t = constp.tile([1, o_out], bf16, name="bias_t")
            nc.scalar.mul(bias_t[:], bias_sb[:], float(t_pool))

            # ---- W^T in bf16 ----
            wl = []
            for l in range(oc):
                wli = wlp.tile([128, c_in], f32, name=f"wl{l}")
                # balance the 4MB of W across both rings so neither ring
                # carries extra bytes and the feature stream tails end together
                eng = nc.sync if l % 2 == 0 else nc.scalar
                eng.dma_start(wli[:], w_in[l * 128:(l + 1) * 128, :])
                wl.append(wli)
            wt = []
            for j in range(nj):
                pswt = psp.tile([128, o_out], f32, name="pswt", tag="rot")
                for l in range(oc):
                    src = wl[l][:, :].rearrange("o (p j) -> o p j", j=nj)[:, :, j]
                    nc.tensor.transpose(pswt[:, l * 128:(l + 1) * 128],
                                        src, identf[:])
                wtj = wtp.tile([128, o_out], bf16, name=f"wt{j}")
                nc.scalar.copy(wtj[:], pswt[:])
                wt.append(wtj)

            gl_full = pp.tile([128, oc, bc], bf16, name="gl_full")
            outsb = pp.tile([bc, b_full], f32, name="outsb")
            glcs = [pp.tile([128, oc * ck], bf16, name=f"glc{c}")
                    for c in range(N_CHUNKS)]
            grts = []

            JSPLIT = 4  # trailing batches of the last chunk streamed j-major

            def pool_chunk(c):
                p4 = lp.tile([128, ck, nj], bf16, name=f"p4_{c}")
                split = JSPLIT if c == N_CHUNKS - 1 else 0
                for i in range(ck - split):
                    b = c * ck + i
                    ft = fp.tile([128, nj * t_pool], f32, name="ft")
                    src = feat[b:b + 1, :, :].rearrange(
                        "b (p j) t -> p (b j t)", j=nj)
                    dma_eng = nc.scalar if b % 2 == 0 else nc.sync
                    dma_eng.dma_start(ft[:], src)
                    with nc.allow_low_precision("pooled sums cast to bf16"):
                        nc.vector.reduce_sum(
                            p4[:, i, :],
                            ft[:].rearrange("p (j t) -> p j t", t=t_pool),
                            axis=X)
                # Stream the last `split` batches j-group-major: once group g
                # of every batch has landed, p4[:, :, 4g:4g+4] is complete and
                # the projection matmuls j = 4g..4g+3 run overlapped with the
                # remaining groups' streaming (only group 3's projection stays
                # exposed after the stream ends).
                n = 0
                for g in range(4):
                    for i in range(ck - split, ck):
                        b = c * ck + i
                        ftj = flp.tile([128, 4, t_pool], f32, name="ftl")
                        eng = nc.scalar if n % 2 == 0 else nc.sync
                        n += 1
                        src = feat[b:b + 1, :, :].rearrange(
                            "b (p j) t -> p (b j) t", j=nj)
                        eng.dma_start(ftj[:], src[:, 4 * g:4 * (g + 1), :])
                        with nc.allow_low_precision("pooled bf16"):
                            nc.vector.reduce_sum(
                                p4[:, i, 4 * g:4 * (g + 1)], ftj[:], axis=X)
                return p4

            def project(c, p4):
                gps = psgp.tile([ck, o_out], f32, name="gps", tag="gps")
                for j in range(nj):
                    nc.tensor.matmul(gps[:], p4[:, :, j], wt[j][:],
                                     start=(j == 0), stop=False)
                nc.tensor.matmul(gps[:], ones[:], bias_t[:],
                                 start=False, stop=True)
                return gps

            def transpose_gn(c, gn):
                glc_v = glcs[c][:].rearrange("p (m i) -> p m i", i=ck)
                for m in range(oc):
                    psg = psp.tile([128, ck], bf16, name="psg", tag="rot")
                    nc.tensor.transpose(psg[:], gn[:, m * 128:(m + 1) * 128],
                                        identb[:])
                    nc.scalar.copy(gl_full[:, m, c * ck:(c + 1) * ck], psg[:])
                    nc.scalar.copy(glc_v[:, m, :], psg[:])

            def allgather(c, dma_eng):
                agin = dp.tile([128, oc * ck], bf16, name=f"agin{c}")
                agout = dp.tile([nr * 128, oc * ck], bf16, name=f"agout{c}",
                                addr_space="Shared")
                dma_eng.dma_start(agin[:], glcs[c][:])
                nc.gpsimd.collective_compute(
                    "AllGather", AL.bypass,
                    replica_groups=[list(range(n_cores))],
                    ins=[agin.opt()], outs=[agout.opt()],
                )
                grt = pp.tile([128, nr, oc * ck], bf16, name=f"grt{c}")
                agv = agout[:, :].rearrange("(r p) f -> p r f", r=nr)
                if dma_eng is nc.gpsimd:
                    dma_eng.dma_start(grt[:], agv)
                else:
                    # exposed load: split across both idle HWDGE rings
                    h = nr // 2
                    nc.sync.dma_start(grt[:, :h, :], agv[:, :h, :])
                    nc.scalar.dma_start(grt[:, h:, :], agv[:, h:, :])
                grts.append(grt)

            # ================= chunk 0 =================
            p4 = pool_chunk(0)
            gps = project(0, p4)
            scr = pp.tile([ck, o_out], f32, name="scr0")
            n2 = pp.tile([ck, 1], f32, name="n20")
            nc.scalar.activation(scr[:], gps[:], AF.Square, accum_out=n2[:])
            gsb = pp.tile([ck, o_out], f32, name="gsb0")
            nc.scalar.copy(gsb[:], gps[:])
            nrm = pp.tile([ck, 1], f32, name="nrm0")
            nc.scalar.sqrt(nrm[:], n2[:])
            gn0 = pp.tile([ck, o_out], bf16, name="gn0")
            nc.gpsimd.normalize_recip(gn0[:], gsb[:], nrm[:])
            transpose_gn(0, gn0)
            allgather(0, nc.gpsimd)

            # ================= chunk 1 =================
            p4 = pool_chunk(1)
            gps = project(1, p4)
            scr1 = pp.tile([ck, o_out], f32, name="scr1")
            n21 = pp.tile([ck, 1], f32, name="n21")
            nc.scalar.activation(scr1[:], gps[:], AF.Square, accum_out=n21[:])
            nrm1 = pp.tile([ck, 1], f32, name="nrm1")
            nc.scalar.sqrt(nrm1[:], n21[:])
            rinv1 = pp.tile([ck, 1], f32, name="rinv1")
            nc.vector.reciprocal(rinv1[:], nrm1[:])
            gn1 = pp.tile([ck, o_out], bf16, name="gn1")
            # scale per 128-block so each transpose starts as soon as its
            # block is written
            for m in range(oc):
                nc.scalar.mul(gn1[:, m * 128:(m + 1) * 128],
                              gps[:, m * 128:(m + 1) * 128], rinv1[:])
            transpose_gn(1, gn1)
            allgather(1, nc.sync)

            # chunk 0 (gathered mid-stream): one [bc, 256] block
            simps = pssp.tile([bc, nr * ck], f32, name="simps0", tag="sim")
            for m in range(oc):
                nc.tensor.matmul(
                    simps[:], gl_full[:, m, :],
                    grts[0][:, :, m * ck:(m + 1) * ck],
                    start=(m == 0), stop=(m == oc - 1))
            dst = outsb[:, :].rearrange(
                "b (r c i) -> b r c i", c=N_CHUNKS, i=ck)[:, :, 0, :]
            nc.vector.tensor_copy(dst, simps[:])
            # chunk 1 (exposed): per rank-half, so each half's matmuls start
            # as soon as its half of the split grt load lands
            hr = nr // 2
            for h in range(2):
                sph = pssp2.tile([bc, hr * ck], f32, name=f"simh{h}",
                                 tag=f"simh{h}")
                for m in range(oc):
                    nc.tensor.matmul(
                        sph[:], gl_full[:, m, :],
                        grts[1][:, h * hr:(h + 1) * hr,
                                m * ck:(m + 1) * ck],
                        start=(m == 0), stop=(m == oc - 1))
                dsth = outsb[:, :].rearrange(
                    "b (r c i) -> b r c i", c=N_CHUNKS, i=ck)[
                        :, h * hr:(h + 1) * hr, 1, :]
                nc.vector.tensor_copy(dsth, sph[:])

            # row-split the output store across both idle rings
            nc.sync.dma_start(out_d[:bc // 2, :], outsb[:bc // 2, :])
            nc.scalar.dma_start(out_d[bc // 2:, :], outsb[bc // 2:, :])

    nc.compile()
    return nc


_NC_CACHE = {}


def _get_nc():
    key = (B_FULL, C_IN, T_POOL, O_OUT, N_CORES)
    if key not in _NC_CACHE:
        _NC_CACHE[key] = build_kernel(*key)
    return _NC_CACHE[key]


def _run(features, W, bias, trace=False, tmpdir=None):
    from concourse.bass_utils import run_bass_kernel_spmd

    feats = np.ascontiguousarray(np.asarray(features, dtype=np.float32))
    w_np = np.ascontiguousarray(np.asarray(W, dtype=np.float32))
    bias_np = np.ascontiguousarray(
        np.asarray(bias, dtype=np.float32).reshape(1, O_OUT))
    bc = B_FULL // N_CORES

    nc = _get_nc()
    in_maps = [
        {"features": feats[r * bc:(r + 1) * bc], "w": w_np, "bias": bias_np}
        for r in range(N_CORES)
    ]
    kw = {"tmpdir": tmpdir} if tmpdir else {}
    res = run_bass_kernel_spmd(nc, in_maps, core_ids=list(range(N_CORES)),
                               trace=trace, **kw)
    out = np.concatenate([res.results[r]["out"] for r in range(N_CORES)], axis=0)
    return out, res.exec_time_ns


def kernel(features, W, bias):
    out, _ = _run(features, W, bias)
    return out

